# revision 16
# baseline (speedup 1.0000x reference)
"""Trainium2 Bass kernel for nn_Attention (dense transformer block:
qkv proj + RoPE + causal attention + out proj), tensor-parallel over
8 NeuronCores: core c handles batch b=c//2, head-group g=c%2 (8 heads).

Self-contained: hardcodes all shapes; host preps transposed/permuted
shards, device computes partial y per core, host sums head-group pairs
and adds the output bias.

All matmul operands are bf16 (1 cyc/row at any width, half the HBM
traffic; accumulation stays fp32 in PSUM, softmax denominators and the
final output stay fp32).  The RoPE partition swap is a permutation
matmul on the tensor engine.  Weights are pre-arranged on the host so
every weight load is one large contiguous DMA.
"""

from contextlib import ExitStack

import numpy as np

import concourse.bass as bass
import concourse.tile as tile
from concourse import bacc, mybir
from concourse.bass import ds, ts
from concourse.bass_utils import run_bass_kernel_spmd

B, S, D, H, DH = 4, 2048, 1024, 16, 64
HL = 8          # heads per core
INNER = H * DH  # 1024
KC = D // 128   # 8 contraction chunks
NT = S // 128   # 16 token tiles
F32 = mybir.dt.float32
BF16 = mybir.dt.bfloat16


def _pieces(cw):
    """split a psum-tile column span into single-bank matmul pieces"""
    out = [(i * 512, 512) for i in range(cw // 512)]
    if cw % 512:
        out.append((cw - cw % 512, cw % 512))
    return out


def build_kernel(nc, phases=3, loop_n=0):
    xT = nc.dram_tensor("xT", [D, S], BF16, kind="ExternalInput").ap()
    # [t, p, k, n]: per q/k col-tile t, contraction chunk k pre-split
    wqk = nc.dram_tensor("wqk", [8, 128, KC, 128], BF16,
                         kind="ExternalInput").ap()
    wv = nc.dram_tensor("wv", [128, KC, 512], BF16, kind="ExternalInput").ap()
    wo = nc.dram_tensor("wo", [128, 4, D], BF16, kind="ExternalInput").ap()
    cc = nc.dram_tensor("cc", [128, S], F32, kind="ExternalInput").ap()
    ssw = nc.dram_tensor("ssw", [128, S], F32, kind="ExternalInput").ap()
    perm = nc.dram_tensor("perm", [128, 128], BF16, kind="ExternalInput").ap()
    y = nc.dram_tensor("y", [S, D], F32, kind="ExternalOutput").ap()

    EXP = mybir.ActivationFunctionType.Exp
    SCALE = 1.0 / np.sqrt(DH)

    with tile.TileContext(nc) as tc, ExitStack() as top:
        if loop_n:
            top.enter_context(tc.For_i(0, loop_n, 1,
                                       hint_engines=(mybir.EngineType.PE,)))
        opool = top.enter_context(tc.tile_pool(name="opool", bufs=1))
        wop = top.enter_context(tc.tile_pool(name="wop", bufs=1))
        ot = [None] * 4

        with ExitStack() as mid:
            qkp = mid.enter_context(tc.tile_pool(name="qkt", bufs=1))
            vpool = mid.enter_context(tc.tile_pool(name="vpool", bufs=1))
            qkt = [qkp.tile([128, S], BF16, tag=f"qkt{t}", name=f"qkt{t}")
                   for t in range(8)]
            vsb = vpool.tile([128, NT, HL, DH + 1], BF16, tag="vsb", name="vsb")

            # ---------------- phase B: projections + rope -----------------
            with ExitStack() as ph:
                consts = ph.enter_context(tc.tile_pool(name="consts", bufs=1))
                xtp = ph.enter_context(tc.tile_pool(name="xtp", bufs=2))
                wsl = ph.enter_context(tc.tile_pool(name="wsl", bufs=2))
                rtmp = ph.enter_context(tc.tile_pool(name="rtmp", bufs=3))
                psA = ph.enter_context(
                    tc.tile_pool(name="psA", bufs=2, space="PSUM"))
                psB = ph.enter_context(
                    tc.tile_pool(name="psB", bufs=1, space="PSUM"))
                psv = ph.enter_context(
                    tc.tile_pool(name="psv", bufs=2, space="PSUM"))

                # x tiles on sync, w tiles on scalar, everything else on the
                # SWDGE queue — emitted AFTER the first x batch so the big
                # const transfers don't hold the DMA engines while the first
                # matmul group waits on x
                cc_sb = consts.tile([128, S], F32, tag="cc", name="cc")
                ssw_sb = consts.tile([128, S], F32, tag="ssw", name="ssw")
                perm_sb = consts.tile([128, 128], BF16, tag="perm", name="perm")
                wv_sb = consts.tile([128, KC, 512], BF16, tag="wv", name="wv")
                wo_sb = wop.tile([128, 4, D], BF16, tag="wo", name="wo")

                for half in range(2):
                    hs = ds(half * 1024, 1024)
                    xth = []
                    for k in range(KC):
                        xh = xtp.tile([128, 1024], BF16, tag=f"xth{k}",
                                      name=f"xth{k}")
                        nc.sync.dma_start(
                            xh[:], xT[ts(k, 128), ds(half * 1024, 1024)])
                        xth.append(xh)
                    if half == 0:
                        # gate the const loads on the first x tile so their
                        # transfers don't queue ahead of the critical-path
                        # x/w tiles on the shared DMA engines; chunk to
                        # <=512KB for fair interleaving
                        gate = consts.tile([1, 8], BF16, tag="gate",
                                           name="gate")
                        nc.gpsimd.tensor_copy(gate[:], xth[0][0:1, 0:8])
                        for q4 in range(4):
                            nc.gpsimd.dma_start(cc_sb[:, ts(q4, 512)],
                                                cc[:, ts(q4, 512)])
                        for q4 in range(4):
                            nc.gpsimd.dma_start(ssw_sb[:, ts(q4, 512)],
                                                ssw[:, ts(q4, 512)])
                        nc.gpsimd.dma_start(perm_sb[:], perm)
                        for wh in range(2):
                            nc.gpsimd.dma_start(wv_sb[:, ts(wh, 4), :],
                                                wv[:, ts(wh, 4), :])
                        nc.gpsimd.dma_start(wo_sb[:], wo)
                        nc.gpsimd.memset(vsb[:, :, :, DH], 1.0)
                    # q/k projections interleaved with v projection
                    for t in range(8):
                        wt = wsl.tile([128, KC, 128], BF16, tag="w", name="w")
                        nc.scalar.dma_start(wt[:], wqk[t])
                        ps = psA.tile([128, 1024], F32, tag="psA")
                        for k in range(KC):
                            for p2 in range(2):
                                nc.tensor.matmul(
                                    ps[:, ts(p2, 512)],
                                    (wt[:, k, :]),
                                    (xth[k][:, ts(p2, 512)]),
                                    start=(k == 0), stop=(k == KC - 1))
                        # rope: qkt = ps*CC + P32swap @ (ps*SSsw)
                        v2 = rtmp.tile([128, 1024], BF16, tag="v2")
                        nc.vector.tensor_mul(v2[:], ps[:], ssw_sb[:, hs])
                        # v projection fills the PE while the DVE mul runs
                        tt = half * 8 + t
                        psV = psv.tile([128, 512], F32, tag="psv")
                        for k in range(KC):
                            nc.tensor.matmul(
                                psV[:], (xth[k][:, ds(t * 128, 128)]),
                                (wv_sb[:, k, :]),
                                start=(k == 0), stop=(k == KC - 1))
                        pb = psB.tile([128, 1024], F32, tag="psB")
                        for p2 in range(2):
                            nc.tensor.matmul(
                                pb[:, ts(p2, 512)], perm_sb[:],
                                v2[:, ts(p2, 512)], start=True, stop=True)
                        t1 = rtmp.tile([128, 1024], BF16, tag="t1")
                        nc.vector.tensor_mul(t1[:], ps[:], cc_sb[:, hs])
                        nc.vector.tensor_tensor(
                            qkt[t][:, hs], t1[:], pb[:],
                            op=mybir.AluOpType.add)
                        nc.scalar.copy(
                            vsb[:, tt, :, 0:DH],
                            psV[:].rearrange("p (h d) -> p h d", h=HL))

            # ---------------- attention ----------------------------------
            if phases < 2:
                return nc
            with ExitStack() as ph:
                ppool = ph.enter_context(tc.tile_pool(name="ppool", bufs=5))
                lpool = ph.enter_context(tc.tile_pool(name="lpool", bufs=2))
                pssc = ph.enter_context(
                    tc.tile_pool(name="pssc", bufs=2, space="PSUM"))
                psav = ph.enter_context(
                    tc.tile_pool(name="psav", bufs=2, space="PSUM"))

                for qh in range(2):
                    for h in range(HL):
                        ht, hb = h // 2, 64 * (h % 2)
                        if ot[ht] is None:
                            ot[ht] = opool.tile([128, S], BF16, tag=f"ot{ht}",
                                                name=f"ot{ht}")
                        q_ap = qkt[ht][ds(hb, 64), :]
                        k_ap = qkt[4 + ht][ds(hb, 64), :]
                        q0, q1 = 1024 * qh, 1024 * (qh + 1)
                        pav = psav.tile([DH + 1, 1024], F32, tag="pav")
                        for j in range(8 * (qh + 1)):
                            gs = max(q0, 128 * j)     # first valid q col
                            cw = q1 - gs
                            ps = pssc.tile([128, cw], F32, tag="sc")
                            for (po, pw) in _pieces(cw):
                                nc.tensor.matmul(
                                    ps[:, ds(po, pw)],
                                    (k_ap[:, ds(128 * j, 128)]),
                                    (q_ap[:, ds(gs + po, pw)]),
                                    start=True, stop=True)
                            pj = ppool.tile([128, cw], BF16, tag="P")
                            nc.scalar.activation(pj[:], ps[:], EXP, scale=SCALE)
                            if gs == 128 * j:
                                # diagonal block: causal-mask first 128 cols
                                nc.gpsimd.affine_select(
                                    out=pj[:, 0:128], in_=pj[:, 0:128],
                                    compare_op=mybir.AluOpType.is_ge, fill=0.0,
                                    base=0, pattern=[[1, 128]],
                                    channel_multiplier=-1)
                            for c in range(max(2 * qh, j // 4), 2 * qh + 2):
                                cs = max(512 * c, 128 * j)
                                w = 512 * (c + 1) - cs
                                nc.tensor.matmul(
                                    pav[:, ds(cs - q0, w)],
                                    (vsb[:, j, h, :]),
                                    (pj[:, ds(cs - gs, w)]),
                                    start=(j == 0),
                                    stop=(j == min(8 * (qh + 1) - 1, 4 * c + 3)))
                        # normalize: ot rows = pav[:64] / l, l = pav[64]
                        qsl = ds(q0, 1024)
                        lr = lpool.tile([128, 1024], F32, tag="lr")
                        nc.vector.tensor_copy(lr[ds(64, 1), :], pav[ds(DH, 1), :])
                        nc.sync.dma_start(lr[ds(0, 1), :], lr[ds(64, 1), :])
                        nc.vector.reciprocal(lr[ds(0, 1), :], lr[ds(0, 1), :])
                        rb = lpool.tile([64, 1024], F32, tag="rb")
                        nc.gpsimd.partition_broadcast(rb[:], lr[ds(0, 1), :],
                                                      channels=64)
                        if h % 2 == 0:
                            nc.vector.tensor_mul(
                                ot[ht][ds(0, 64), qsl], pav[ds(0, DH), :], rb[:])
                        else:
                            ott = lpool.tile([64, 1024], BF16, tag="ott")
                            nc.vector.tensor_mul(ott[:], pav[ds(0, DH), :], rb[:])
                            nc.sync.dma_start(ot[ht][ds(64, 64), qsl], ott[:])

        # ---------------- out projection ---------------------------------
        if phases < 3:
            return nc
        with ExitStack() as ph:
            ypool = ph.enter_context(tc.tile_pool(name="ypool", bufs=3))
            psy = ph.enter_context(
                tc.tile_pool(name="psy", bufs=2, space="PSUM"))
            for tt in range(NT):
                ps = psy.tile([128, D], F32, tag="psy")
                for k in range(4):
                    for half in range(2):
                        nc.tensor.matmul(
                            ps[:, ts(half, 512)],
                            (ot[k][:, ts(tt, 128)]),
                            (wo_sb[:, k, ts(half, 512)]),
                            start=(k == 0), stop=(k == 3))
                ysb = ypool.tile([128, D], F32, tag="y")
                nc.scalar.copy(ysb[:], ps[:])
                nc.sync.dma_start(y[ts(tt, 128), :], ysb[:])
    return nc


# ---------------- host side ------------------------------------------------

def _bf16(a):
    import ml_dtypes
    return np.ascontiguousarray(np.asarray(a).astype(ml_dtypes.bfloat16))


def _rope_tables():
    i = np.arange(DH // 2, dtype=np.float32)
    thetas = np.power(np.float32(10000.0), -2.0 * (i - 1.0) / DH)
    vals = thetas[:, None].astype(np.float32) * \
        np.arange(S, dtype=np.float32)[None, :]
    cos32 = np.cos(vals).astype(np.float32)
    sin32 = np.sin(vals).astype(np.float32)
    CC = np.tile(cos32, (4, 1))
    SSsw = np.concatenate([sin32, -sin32, sin32, -sin32], axis=0)
    return np.ascontiguousarray(CC), np.ascontiguousarray(SSsw)


def _perm_matrix():
    P = np.zeros((128, 128), dtype=np.float32)
    for m in range(128):
        P[m ^ 32, m] = 1.0
    return P


def _qk_col_perm(g):
    cols = []
    for m in range(4):
        for hh in (2 * m, 2 * m + 1):
            hg = HL * g + hh
            cols += [hg * DH + 2 * i for i in range(32)]
            cols += [hg * DH + 2 * i + 1 for i in range(32)]
    return np.array(cols)


_CACHE = {}


def _get_module():
    if "nc" not in _CACHE:
        nc = bacc.Bacc("TRN2", target_bir_lowering=False, debug=False,
                       num_devices=8)
        build_kernel(nc)
        nc.compile()
        _CACHE["nc"] = nc
    return _CACHE["nc"]


def make_in_maps(x, Wqkv, Wout):
    x = np.ascontiguousarray(np.asarray(x, np.float32))
    Wqkv = np.ascontiguousarray(np.asarray(Wqkv, np.float32))
    Wout = np.ascontiguousarray(np.asarray(Wout, np.float32))
    CC, SSsw = _rope_tables()
    P = _bf16(_perm_matrix())
    shard = {}
    for g in range(2):
        perm = _qk_col_perm(g)
        vcols = np.arange(HL * g * DH, HL * (g + 1) * DH)
        wqp = Wqkv[:, 0 * INNER:1 * INNER][:, perm]
        wkp = Wqkv[:, 1 * INNER:2 * INNER][:, perm]
        # [t, p, k, n]: tile t's [D, 128] block split into 8 [128, 128]
        # contraction chunks, partition-major
        wqk = np.stack([
            (wqp if t < 4 else wkp)[:, (t % 4) * 128:(t % 4 + 1) * 128]
            .reshape(KC, 128, 128).transpose(1, 0, 2)
            for t in range(8)])
        shard[g] = dict(
            wqk=_bf16(wqk),
            wv=_bf16(Wqkv[:, 2 * INNER:3 * INNER][:, vcols]
                     .reshape(KC, 128, 512).transpose(1, 0, 2)),
            wo=_bf16(Wout[vcols, :].reshape(4, 128, D).transpose(1, 0, 2)),
        )
    in_maps = []
    for c in range(8):
        b, g = c // 2, c % 2
        in_maps.append(dict(
            xT=_bf16(x[b].T), cc=CC, ssw=SSsw, perm=P, **shard[g]))
    return in_maps


def kernel(x, Wqkv, Wout, bout):
    bout = np.asarray(bout, np.float32)
    nc = _get_module()
    in_maps = make_in_maps(x, Wqkv, Wout)
    res = run_bass_kernel_spmd(nc, in_maps, core_ids=list(range(8)))
    ys = [r["y"] for r in res.results]
    out = np.stack([ys[2 * b] + ys[2 * b + 1] + bout for b in range(B)])
    return out.astype(np.float32)


# revision 17
# speedup vs baseline: 1.0272x; 1.0272x over previous
"""Trainium2 Bass kernel for nn_Attention (dense transformer block:
qkv proj + RoPE + causal attention + out proj), tensor-parallel over
8 NeuronCores: core c handles batch b=c//2, head-group g=c%2 (8 heads).

Self-contained: hardcodes all shapes; host preps transposed/permuted
shards, device computes partial y per core, host sums head-group pairs
and adds the output bias.

All matmul operands are bf16 (1 cyc/row at any width, half the HBM
traffic; accumulation stays fp32 in PSUM, softmax denominators and the
final output stay fp32).  The RoPE partition swap is a permutation
matmul on the tensor engine.  Weights are pre-arranged on the host so
every weight load is one large contiguous DMA.
"""

from contextlib import ExitStack

import numpy as np

import concourse.bass as bass
import concourse.tile as tile
from concourse import bacc, mybir
from concourse.bass import ds, ts
from concourse.bass_utils import run_bass_kernel_spmd

B, S, D, H, DH = 4, 2048, 1024, 16, 64
HL = 8          # heads per core
INNER = H * DH  # 1024
KC = D // 128   # 8 contraction chunks
NT = S // 128   # 16 token tiles
F32 = mybir.dt.float32
BF16 = mybir.dt.bfloat16


def _pieces(cw):
    """split a psum-tile column span into single-bank matmul pieces"""
    out = [(i * 512, 512) for i in range(cw // 512)]
    if cw % 512:
        out.append((cw - cw % 512, cw % 512))
    return out


def build_kernel(nc, phases=3, loop_n=0):
    xT = nc.dram_tensor("xT", [D, S], BF16, kind="ExternalInput").ap()
    # [t, p, k, n]: per q/k col-tile t, contraction chunk k pre-split
    wqk = nc.dram_tensor("wqk", [8, 128, KC, 128], BF16,
                         kind="ExternalInput").ap()
    wv = nc.dram_tensor("wv", [128, KC, 512], BF16, kind="ExternalInput").ap()
    wo = nc.dram_tensor("wo", [128, 4, D], BF16, kind="ExternalInput").ap()
    cc = nc.dram_tensor("cc", [128, S], F32, kind="ExternalInput").ap()
    ssw = nc.dram_tensor("ssw", [128, S], F32, kind="ExternalInput").ap()
    perm = nc.dram_tensor("perm", [128, 128], BF16, kind="ExternalInput").ap()
    y = nc.dram_tensor("y", [S, D], F32, kind="ExternalOutput").ap()

    EXP = mybir.ActivationFunctionType.Exp
    SCALE = 1.0 / np.sqrt(DH)

    with tile.TileContext(nc) as tc, ExitStack() as top:
        if loop_n:
            top.enter_context(tc.For_i(0, loop_n, 1,
                                       hint_engines=(mybir.EngineType.PE,)))
        opool = top.enter_context(tc.tile_pool(name="opool", bufs=1))
        wop = top.enter_context(tc.tile_pool(name="wop", bufs=1))
        ot = [None] * 4

        with ExitStack() as mid:
            qkp = mid.enter_context(tc.tile_pool(name="qkt", bufs=1))
            vpool = mid.enter_context(tc.tile_pool(name="vpool", bufs=1))
            qkt = [qkp.tile([128, S], BF16, tag=f"qkt{t}", name=f"qkt{t}")
                   for t in range(8)]
            vsb = vpool.tile([128, NT, HL, DH + 1], BF16, tag="vsb", name="vsb")

            # ---------------- phase B: projections + rope -----------------
            with ExitStack() as ph:
                consts = ph.enter_context(tc.tile_pool(name="consts", bufs=1))
                xtp = ph.enter_context(tc.tile_pool(name="xtp", bufs=2))
                wsl = ph.enter_context(tc.tile_pool(name="wsl", bufs=3))
                rtmp = ph.enter_context(tc.tile_pool(name="rtmp", bufs=4))
                psA = ph.enter_context(
                    tc.tile_pool(name="psA", bufs=2, space="PSUM"))
                psB = ph.enter_context(
                    tc.tile_pool(name="psB", bufs=1, space="PSUM"))
                psv = ph.enter_context(
                    tc.tile_pool(name="psv", bufs=2, space="PSUM"))

                # x tiles on sync, w tiles on scalar, everything else on the
                # SWDGE queue — emitted AFTER the first x batch so the big
                # const transfers don't hold the DMA engines while the first
                # matmul group waits on x
                cc_sb = consts.tile([128, S], F32, tag="cc", name="cc")
                ssw_sb = consts.tile([128, S], F32, tag="ssw", name="ssw")
                perm_sb = consts.tile([128, 128], BF16, tag="perm", name="perm")
                wv_sb = consts.tile([128, KC, 512], BF16, tag="wv", name="wv")
                wo_sb = wop.tile([128, 4, D], BF16, tag="wo", name="wo")

                for half in range(2):
                    hs = ds(half * 1024, 1024)
                    xth = []
                    for k in range(KC):
                        xh = xtp.tile([128, 1024], BF16, tag=f"xth{k}",
                                      name=f"xth{k}")
                        nc.sync.dma_start(
                            xh[:], xT[ts(k, 128), ds(half * 1024, 1024)])
                        xth.append(xh)
                    if half == 0:
                        # gate the const loads on the first x tile so their
                        # transfers don't queue ahead of the critical-path
                        # x/w tiles on the shared DMA engines; chunk to
                        # <=512KB for fair interleaving
                        gate = consts.tile([1, 8], BF16, tag="gate",
                                           name="gate")
                        nc.gpsimd.tensor_copy(gate[:], xth[0][0:1, 0:8])
                        for q4 in range(4):
                            nc.gpsimd.dma_start(cc_sb[:, ts(q4, 512)],
                                                cc[:, ts(q4, 512)])
                        for q4 in range(4):
                            nc.gpsimd.dma_start(ssw_sb[:, ts(q4, 512)],
                                                ssw[:, ts(q4, 512)])
                        nc.gpsimd.dma_start(perm_sb[:], perm)
                        for wh in range(2):
                            nc.gpsimd.dma_start(wv_sb[:, ts(wh, 4), :],
                                                wv[:, ts(wh, 4), :])
                        nc.gpsimd.dma_start(wo_sb[:], wo)
                        nc.gpsimd.memset(vsb[:, :, :, DH], 1.0)
                    # q/k projections interleaved with v projection
                    for t in range(8):
                        wt = wsl.tile([128, KC, 128], BF16, tag="w", name="w")
                        nc.scalar.dma_start(wt[:], wqk[t])
                        ps = psA.tile([128, 1024], F32, tag="psA")
                        for k in range(KC):
                            for p2 in range(2):
                                nc.tensor.matmul(
                                    ps[:, ts(p2, 512)],
                                    (wt[:, k, :]),
                                    (xth[k][:, ts(p2, 512)]),
                                    start=(k == 0), stop=(k == KC - 1))
                        # rope: qkt = ps*CC + P32swap @ (ps*SSsw)
                        v2 = rtmp.tile([128, 1024], BF16, tag="v2")
                        nc.vector.tensor_mul(v2[:], ps[:], ssw_sb[:, hs])
                        # v projection fills the PE while the DVE mul runs
                        tt = half * 8 + t
                        psV = psv.tile([128, 512], F32, tag="psv")
                        for k in range(KC):
                            nc.tensor.matmul(
                                psV[:], (xth[k][:, ds(t * 128, 128)]),
                                (wv_sb[:, k, :]),
                                start=(k == 0), stop=(k == KC - 1))
                        pb = psB.tile([128, 1024], F32, tag="psB")
                        for p2 in range(2):
                            nc.tensor.matmul(
                                pb[:, ts(p2, 512)], perm_sb[:],
                                v2[:, ts(p2, 512)], start=True, stop=True)
                        t1 = rtmp.tile([128, 1024], BF16, tag="t1")
                        nc.vector.tensor_mul(t1[:], ps[:], cc_sb[:, hs])
                        nc.vector.tensor_tensor(
                            qkt[t][:, hs], t1[:], pb[:],
                            op=mybir.AluOpType.add)
                        nc.scalar.copy(
                            vsb[:, tt, :, 0:DH],
                            psV[:].rearrange("p (h d) -> p h d", h=HL))

            # ---------------- attention ----------------------------------
            if phases < 2:
                return nc
            with ExitStack() as ph:
                ppool = ph.enter_context(tc.tile_pool(name="ppool", bufs=8))
                lpool = ph.enter_context(tc.tile_pool(name="lpool", bufs=4))
                pssc = ph.enter_context(
                    tc.tile_pool(name="pssc", bufs=2, space="PSUM"))
                psav = ph.enter_context(
                    tc.tile_pool(name="psav", bufs=2, space="PSUM"))

                for qh in range(2):
                    for h in range(HL):
                        ht, hb = h // 2, 64 * (h % 2)
                        if ot[ht] is None:
                            ot[ht] = opool.tile([128, S], BF16, tag=f"ot{ht}",
                                                name=f"ot{ht}")
                        q_ap = qkt[ht][ds(hb, 64), :]
                        k_ap = qkt[4 + ht][ds(hb, 64), :]
                        q0, q1 = 1024 * qh, 1024 * (qh + 1)
                        pav = psav.tile([DH + 1, 1024], F32, tag="pav")
                        for j in range(8 * (qh + 1)):
                            gs = max(q0, 128 * j)     # first valid q col
                            cw = q1 - gs
                            ps = pssc.tile([128, cw], F32, tag="sc")
                            for (po, pw) in _pieces(cw):
                                nc.tensor.matmul(
                                    ps[:, ds(po, pw)],
                                    (k_ap[:, ds(128 * j, 128)]),
                                    (q_ap[:, ds(gs + po, pw)]),
                                    start=True, stop=True)
                            pj = ppool.tile([128, cw], BF16, tag="P")
                            nc.scalar.activation(pj[:], ps[:], EXP, scale=SCALE)
                            if gs == 128 * j:
                                # diagonal block: causal-mask first 128 cols
                                nc.gpsimd.affine_select(
                                    out=pj[:, 0:128], in_=pj[:, 0:128],
                                    compare_op=mybir.AluOpType.is_ge, fill=0.0,
                                    base=0, pattern=[[1, 128]],
                                    channel_multiplier=-1)
                            for c in range(max(2 * qh, j // 4), 2 * qh + 2):
                                cs = max(512 * c, 128 * j)
                                w = 512 * (c + 1) - cs
                                nc.tensor.matmul(
                                    pav[:, ds(cs - q0, w)],
                                    (vsb[:, j, h, :]),
                                    (pj[:, ds(cs - gs, w)]),
                                    start=(j == 0),
                                    stop=(j == min(8 * (qh + 1) - 1, 4 * c + 3)))
                        # normalize: ot rows = pav[:64] / l, l = pav[64]
                        qsl = ds(q0, 1024)
                        lr = lpool.tile([128, 1024], F32, tag="lr")
                        nc.vector.tensor_copy(lr[ds(64, 1), :], pav[ds(DH, 1), :])
                        nc.sync.dma_start(lr[ds(0, 1), :], lr[ds(64, 1), :])
                        nc.vector.reciprocal(lr[ds(0, 1), :], lr[ds(0, 1), :])
                        rb = lpool.tile([64, 1024], F32, tag="rb")
                        nc.gpsimd.partition_broadcast(rb[:], lr[ds(0, 1), :],
                                                      channels=64)
                        if h % 2 == 0:
                            nc.vector.tensor_mul(
                                ot[ht][ds(0, 64), qsl], pav[ds(0, DH), :], rb[:])
                        else:
                            ott = lpool.tile([64, 1024], BF16, tag="ott")
                            nc.vector.tensor_mul(ott[:], pav[ds(0, DH), :], rb[:])
                            nc.sync.dma_start(ot[ht][ds(64, 64), qsl], ott[:])

        # ---------------- out projection ---------------------------------
        if phases < 3:
            return nc
        with ExitStack() as ph:
            ypool = ph.enter_context(tc.tile_pool(name="ypool", bufs=4))
            psy = ph.enter_context(
                tc.tile_pool(name="psy", bufs=2, space="PSUM"))
            for tt in range(NT):
                ps = psy.tile([128, D], F32, tag="psy")
                for k in range(4):
                    for half in range(2):
                        nc.tensor.matmul(
                            ps[:, ts(half, 512)],
                            (ot[k][:, ts(tt, 128)]),
                            (wo_sb[:, k, ts(half, 512)]),
                            start=(k == 0), stop=(k == 3))
                ysb = ypool.tile([128, D], F32, tag="y")
                nc.scalar.copy(ysb[:], ps[:])
                nc.sync.dma_start(y[ts(tt, 128), :], ysb[:])
    return nc


# ---------------- host side ------------------------------------------------

def _bf16(a):
    import ml_dtypes
    return np.ascontiguousarray(np.asarray(a).astype(ml_dtypes.bfloat16))


def _rope_tables():
    i = np.arange(DH // 2, dtype=np.float32)
    thetas = np.power(np.float32(10000.0), -2.0 * (i - 1.0) / DH)
    vals = thetas[:, None].astype(np.float32) * \
        np.arange(S, dtype=np.float32)[None, :]
    cos32 = np.cos(vals).astype(np.float32)
    sin32 = np.sin(vals).astype(np.float32)
    CC = np.tile(cos32, (4, 1))
    SSsw = np.concatenate([sin32, -sin32, sin32, -sin32], axis=0)
    return np.ascontiguousarray(CC), np.ascontiguousarray(SSsw)


def _perm_matrix():
    P = np.zeros((128, 128), dtype=np.float32)
    for m in range(128):
        P[m ^ 32, m] = 1.0
    return P


def _qk_col_perm(g):
    cols = []
    for m in range(4):
        for hh in (2 * m, 2 * m + 1):
            hg = HL * g + hh
            cols += [hg * DH + 2 * i for i in range(32)]
            cols += [hg * DH + 2 * i + 1 for i in range(32)]
    return np.array(cols)


_CACHE = {}


def _get_module():
    if "nc" not in _CACHE:
        nc = bacc.Bacc("TRN2", target_bir_lowering=False, debug=False,
                       num_devices=8)
        build_kernel(nc)
        nc.compile()
        _CACHE["nc"] = nc
    return _CACHE["nc"]


def make_in_maps(x, Wqkv, Wout):
    x = np.ascontiguousarray(np.asarray(x, np.float32))
    Wqkv = np.ascontiguousarray(np.asarray(Wqkv, np.float32))
    Wout = np.ascontiguousarray(np.asarray(Wout, np.float32))
    CC, SSsw = _rope_tables()
    P = _bf16(_perm_matrix())
    shard = {}
    for g in range(2):
        perm = _qk_col_perm(g)
        vcols = np.arange(HL * g * DH, HL * (g + 1) * DH)
        wqp = Wqkv[:, 0 * INNER:1 * INNER][:, perm]
        wkp = Wqkv[:, 1 * INNER:2 * INNER][:, perm]
        # [t, p, k, n]: tile t's [D, 128] block split into 8 [128, 128]
        # contraction chunks, partition-major
        wqk = np.stack([
            (wqp if t < 4 else wkp)[:, (t % 4) * 128:(t % 4 + 1) * 128]
            .reshape(KC, 128, 128).transpose(1, 0, 2)
            for t in range(8)])
        shard[g] = dict(
            wqk=_bf16(wqk),
            wv=_bf16(Wqkv[:, 2 * INNER:3 * INNER][:, vcols]
                     .reshape(KC, 128, 512).transpose(1, 0, 2)),
            wo=_bf16(Wout[vcols, :].reshape(4, 128, D).transpose(1, 0, 2)),
        )
    in_maps = []
    for c in range(8):
        b, g = c // 2, c % 2
        in_maps.append(dict(
            xT=_bf16(x[b].T), cc=CC, ssw=SSsw, perm=P, **shard[g]))
    return in_maps


def kernel(x, Wqkv, Wout, bout):
    bout = np.asarray(bout, np.float32)
    nc = _get_module()
    in_maps = make_in_maps(x, Wqkv, Wout)
    res = run_bass_kernel_spmd(nc, in_maps, core_ids=list(range(8)))
    ys = [r["y"] for r in res.results]
    out = np.stack([ys[2 * b] + ys[2 * b + 1] + bout for b in range(B)])
    return out.astype(np.float32)


# revision 18
# speedup vs baseline: 1.0466x; 1.0189x over previous
"""Trainium2 Bass kernel for nn_Attention (dense transformer block:
qkv proj + RoPE + causal attention + out proj), tensor-parallel over
8 NeuronCores: core c handles batch b=c//2, head-group g=c%2 (8 heads).

Self-contained: hardcodes all shapes; host preps transposed/permuted
shards, device computes partial y per core, host sums head-group pairs
and adds the output bias.

All matmul operands are bf16 (1 cyc/row at any width, half the HBM
traffic; accumulation stays fp32 in PSUM, softmax denominators and the
final output stay fp32).  The RoPE partition swap is a permutation
matmul on the tensor engine.  Weights are pre-arranged on the host so
every weight load is one large contiguous DMA.
"""

from contextlib import ExitStack

import numpy as np

import concourse.bass as bass
import concourse.tile as tile
from concourse import bacc, mybir
from concourse.bass import ds, ts
from concourse.bass_utils import run_bass_kernel_spmd

B, S, D, H, DH = 4, 2048, 1024, 16, 64
HL = 8          # heads per core
INNER = H * DH  # 1024
KC = D // 128   # 8 contraction chunks
NT = S // 128   # 16 token tiles
F32 = mybir.dt.float32
BF16 = mybir.dt.bfloat16


def _pieces(cw):
    """split a psum-tile column span into single-bank matmul pieces"""
    out = [(i * 512, 512) for i in range(cw // 512)]
    if cw % 512:
        out.append((cw - cw % 512, cw % 512))
    return out


def build_kernel(nc, phases=3, loop_n=0):
    xT = nc.dram_tensor("xT", [D, S], BF16, kind="ExternalInput").ap()
    # [t, p, k, n]: per q/k col-tile t, contraction chunk k pre-split
    wqk = nc.dram_tensor("wqk", [8, 128, KC, 128], BF16,
                         kind="ExternalInput").ap()
    wv = nc.dram_tensor("wv", [128, KC, 512], BF16, kind="ExternalInput").ap()
    wo = nc.dram_tensor("wo", [128, 4, D], BF16, kind="ExternalInput").ap()
    cc = nc.dram_tensor("cc", [128, S], F32, kind="ExternalInput").ap()
    ssw = nc.dram_tensor("ssw", [128, S], F32, kind="ExternalInput").ap()
    perm = nc.dram_tensor("perm", [128, 128], BF16, kind="ExternalInput").ap()
    y = nc.dram_tensor("y", [S, D], F32, kind="ExternalOutput").ap()

    EXP = mybir.ActivationFunctionType.Exp
    SCALE = 1.0 / np.sqrt(DH)

    with tile.TileContext(nc) as tc, ExitStack() as top:
        if loop_n:
            top.enter_context(tc.For_i(0, loop_n, 1,
                                       hint_engines=(mybir.EngineType.PE,)))
        opool = top.enter_context(tc.tile_pool(name="opool", bufs=1))
        wop = top.enter_context(tc.tile_pool(name="wop", bufs=1))
        ot = [None] * 4

        with ExitStack() as mid:
            qkp = mid.enter_context(tc.tile_pool(name="qkt", bufs=1))
            vpool = mid.enter_context(tc.tile_pool(name="vpool", bufs=1))
            qkt = [qkp.tile([128, S], BF16, tag=f"qkt{t}", name=f"qkt{t}")
                   for t in range(8)]
            vsb = vpool.tile([128, NT, HL, DH + 1], BF16, tag="vsb", name="vsb")

            # ---------------- phase B: projections + rope -----------------
            with ExitStack() as ph:
                consts = ph.enter_context(tc.tile_pool(name="consts", bufs=1))
                xtp = ph.enter_context(tc.tile_pool(name="xtp", bufs=2))
                wsl = ph.enter_context(tc.tile_pool(name="wsl", bufs=3))
                rtmp = ph.enter_context(tc.tile_pool(name="rtmp", bufs=4))
                psA = ph.enter_context(
                    tc.tile_pool(name="psA", bufs=2, space="PSUM"))
                psB = ph.enter_context(
                    tc.tile_pool(name="psB", bufs=1, space="PSUM"))
                psv = ph.enter_context(
                    tc.tile_pool(name="psv", bufs=2, space="PSUM"))

                # x tiles on sync, w tiles on scalar, everything else on the
                # SWDGE queue — emitted AFTER the first x batch so the big
                # const transfers don't hold the DMA engines while the first
                # matmul group waits on x
                cc_sb = consts.tile([128, S], F32, tag="cc", name="cc")
                ssw_sb = consts.tile([128, S], F32, tag="ssw", name="ssw")
                perm_sb = consts.tile([128, 128], BF16, tag="perm", name="perm")
                wv_sb = consts.tile([128, KC, 512], BF16, tag="wv", name="wv")
                wo_sb = wop.tile([128, 4, D], BF16, tag="wo", name="wo")

                for half in range(2):
                    hs = ds(half * 1024, 1024)
                    xth = []
                    for k in range(KC):
                        xh = xtp.tile([128, 1024], BF16, tag=f"xth{k}",
                                      name=f"xth{k}")
                        nc.sync.dma_start(
                            xh[:], xT[ts(k, 128), ds(half * 1024, 1024)])
                        xth.append(xh)
                    if half == 0:
                        # gate the const loads on the first x tile so their
                        # transfers don't queue ahead of the critical-path
                        # x/w tiles on the shared DMA engines; chunk to
                        # <=512KB for fair interleaving
                        gate = consts.tile([1, 8], BF16, tag="gate",
                                           name="gate")
                        nc.gpsimd.tensor_copy(gate[:], xth[0][0:1, 0:8])
                        for q4 in range(4):
                            nc.gpsimd.dma_start(cc_sb[:, ts(q4, 512)],
                                                cc[:, ts(q4, 512)])
                        for q4 in range(4):
                            nc.gpsimd.dma_start(ssw_sb[:, ts(q4, 512)],
                                                ssw[:, ts(q4, 512)])
                        nc.gpsimd.dma_start(perm_sb[:], perm)
                        for wh in range(2):
                            nc.gpsimd.dma_start(wv_sb[:, ts(wh, 4), :],
                                                wv[:, ts(wh, 4), :])
                        nc.gpsimd.dma_start(wo_sb[:], wo)
                        nc.gpsimd.memset(vsb[:, :, :, DH], 1.0)
                    # q/k projections interleaved with v projection
                    for t in range(8):
                        wt = wsl.tile([128, KC, 128], BF16, tag="w", name="w")
                        nc.scalar.dma_start(wt[:], wqk[t])
                        ps = psA.tile([128, 1024], F32, tag="psA")
                        for k in range(KC):
                            for p2 in range(2):
                                nc.tensor.matmul(
                                    ps[:, ts(p2, 512)],
                                    (wt[:, k, :]),
                                    (xth[k][:, ts(p2, 512)]),
                                    start=(k == 0), stop=(k == KC - 1))
                        # rope: qkt = ps*CC + P32swap @ (ps*SSsw)
                        v2 = rtmp.tile([128, 1024], BF16, tag="v2")
                        nc.vector.tensor_mul(v2[:], ps[:], ssw_sb[:, hs])
                        # v projection fills the PE while the DVE mul runs
                        tt = half * 8 + t
                        psV = psv.tile([128, 512], F32, tag="psv")
                        for k in range(KC):
                            nc.tensor.matmul(
                                psV[:], (xth[k][:, ds(t * 128, 128)]),
                                (wv_sb[:, k, :]),
                                start=(k == 0), stop=(k == KC - 1))
                        pb = psB.tile([128, 1024], F32, tag="psB")
                        for p2 in range(2):
                            nc.tensor.matmul(
                                pb[:, ts(p2, 512)], perm_sb[:],
                                v2[:, ts(p2, 512)], start=True, stop=True)
                        t1 = rtmp.tile([128, 1024], BF16, tag="t1")
                        nc.vector.tensor_mul(t1[:], ps[:], cc_sb[:, hs])
                        nc.vector.tensor_tensor(
                            qkt[t][:, hs], t1[:], pb[:],
                            op=mybir.AluOpType.add)
                        nc.scalar.copy(
                            vsb[:, tt, :, 0:DH],
                            psV[:].rearrange("p (h d) -> p h d", h=HL))

            # ---------------- attention ----------------------------------
            if phases < 2:
                return nc
            with ExitStack() as ph:
                ppool = ph.enter_context(tc.tile_pool(name="ppool", bufs=8))
                lpool = ph.enter_context(tc.tile_pool(name="lpool", bufs=4))
                pssc = ph.enter_context(
                    tc.tile_pool(name="pssc", bufs=2, space="PSUM"))
                psav = ph.enter_context(
                    tc.tile_pool(name="psav", bufs=2, space="PSUM"))

                for qh in range(2):
                    for h in range(HL):
                        ht, hb = h // 2, 64 * (h % 2)
                        if ot[ht] is None:
                            ot[ht] = opool.tile([128, S], BF16, tag=f"ot{ht}",
                                                name=f"ot{ht}")
                        q_ap = qkt[ht][ds(hb, 64), :]
                        k_ap = qkt[4 + ht][ds(hb, 64), :]
                        q0, q1 = 1024 * qh, 1024 * (qh + 1)
                        pav = psav.tile([DH + 1, 1024], F32, tag="pav")
                        for j in range(8 * (qh + 1)):
                            gs = max(q0, 128 * j)     # first valid q col
                            cw = q1 - gs
                            ps = pssc.tile([128, cw], F32, tag="sc")
                            for (po, pw) in _pieces(cw):
                                nc.tensor.matmul(
                                    ps[:, ds(po, pw)],
                                    (k_ap[:, ds(128 * j, 128)]),
                                    (q_ap[:, ds(gs + po, pw)]),
                                    start=True, stop=True)
                            pj = ppool.tile([128, cw], BF16, tag="P")
                            nc.scalar.activation(pj[:], ps[:], EXP, scale=SCALE)
                            if gs == 128 * j:
                                # diagonal block: causal-mask first 128 cols
                                nc.gpsimd.affine_select(
                                    out=pj[:, 0:128], in_=pj[:, 0:128],
                                    compare_op=mybir.AluOpType.is_ge, fill=0.0,
                                    base=0, pattern=[[1, 128]],
                                    channel_multiplier=-1)
                            for c in range(max(2 * qh, j // 4), 2 * qh + 2):
                                cs = max(512 * c, 128 * j)
                                w = 512 * (c + 1) - cs
                                nc.tensor.matmul(
                                    pav[:, ds(cs - q0, w)],
                                    (vsb[:, j, h, :]),
                                    (pj[:, ds(cs - gs, w)]),
                                    start=(j == 0),
                                    stop=(j == min(8 * (qh + 1) - 1, 4 * c + 3)))
                        # normalize: ot rows = pav[:64] / l, l = pav[64]
                        qsl = ds(q0, 1024)
                        lr = lpool.tile([128, 1024], F32, tag="lr")
                        nc.vector.tensor_copy(lr[ds(64, 1), :], pav[ds(DH, 1), :])
                        nc.sync.dma_start(lr[ds(0, 1), :], lr[ds(64, 1), :])
                        # ~5x faster than InstReciprocal (which runs 8 cyc/elem
                        # on a single lane here); 18-bit accuracy is plenty for
                        # the softmax denominator. Input/output are the proven
                        # SBUF row-0 path.
                        nc.vector.reciprocal_approx_fast(
                            out=lr[ds(0, 1), :], in_=lr[ds(0, 1), :])
                        rb = lpool.tile([64, 1024], F32, tag="rb")
                        nc.gpsimd.partition_broadcast(rb[:], lr[ds(0, 1), :],
                                                      channels=64)
                        if h % 2 == 0:
                            nc.vector.tensor_mul(
                                ot[ht][ds(0, 64), qsl], pav[ds(0, DH), :], rb[:])
                        else:
                            ott = lpool.tile([64, 1024], BF16, tag="ott")
                            nc.vector.tensor_mul(ott[:], pav[ds(0, DH), :], rb[:])
                            nc.sync.dma_start(ot[ht][ds(64, 64), qsl], ott[:])

        # ---------------- out projection ---------------------------------
        if phases < 3:
            return nc
        with ExitStack() as ph:
            ypool = ph.enter_context(tc.tile_pool(name="ypool", bufs=4))
            psy = ph.enter_context(
                tc.tile_pool(name="psy", bufs=2, space="PSUM"))
            for tt in range(NT):
                ps = psy.tile([128, D], F32, tag="psy")
                for k in range(4):
                    for half in range(2):
                        nc.tensor.matmul(
                            ps[:, ts(half, 512)],
                            (ot[k][:, ts(tt, 128)]),
                            (wo_sb[:, k, ts(half, 512)]),
                            start=(k == 0), stop=(k == 3))
                ysb = ypool.tile([128, D], F32, tag="y")
                nc.scalar.copy(ysb[:], ps[:])
                nc.sync.dma_start(y[ts(tt, 128), :], ysb[:])
    return nc


# ---------------- host side ------------------------------------------------

def _bf16(a):
    import ml_dtypes
    return np.ascontiguousarray(np.asarray(a).astype(ml_dtypes.bfloat16))


def _rope_tables():
    i = np.arange(DH // 2, dtype=np.float32)
    thetas = np.power(np.float32(10000.0), -2.0 * (i - 1.0) / DH)
    vals = thetas[:, None].astype(np.float32) * \
        np.arange(S, dtype=np.float32)[None, :]
    cos32 = np.cos(vals).astype(np.float32)
    sin32 = np.sin(vals).astype(np.float32)
    CC = np.tile(cos32, (4, 1))
    SSsw = np.concatenate([sin32, -sin32, sin32, -sin32], axis=0)
    return np.ascontiguousarray(CC), np.ascontiguousarray(SSsw)


def _perm_matrix():
    P = np.zeros((128, 128), dtype=np.float32)
    for m in range(128):
        P[m ^ 32, m] = 1.0
    return P


def _qk_col_perm(g):
    cols = []
    for m in range(4):
        for hh in (2 * m, 2 * m + 1):
            hg = HL * g + hh
            cols += [hg * DH + 2 * i for i in range(32)]
            cols += [hg * DH + 2 * i + 1 for i in range(32)]
    return np.array(cols)


_CACHE = {}


def _get_module():
    if "nc" not in _CACHE:
        nc = bacc.Bacc("TRN2", target_bir_lowering=False, debug=False,
                       num_devices=8)
        build_kernel(nc)
        nc.compile()
        _CACHE["nc"] = nc
    return _CACHE["nc"]


def make_in_maps(x, Wqkv, Wout):
    x = np.ascontiguousarray(np.asarray(x, np.float32))
    Wqkv = np.ascontiguousarray(np.asarray(Wqkv, np.float32))
    Wout = np.ascontiguousarray(np.asarray(Wout, np.float32))
    CC, SSsw = _rope_tables()
    P = _bf16(_perm_matrix())
    shard = {}
    for g in range(2):
        perm = _qk_col_perm(g)
        vcols = np.arange(HL * g * DH, HL * (g + 1) * DH)
        wqp = Wqkv[:, 0 * INNER:1 * INNER][:, perm]
        wkp = Wqkv[:, 1 * INNER:2 * INNER][:, perm]
        # [t, p, k, n]: tile t's [D, 128] block split into 8 [128, 128]
        # contraction chunks, partition-major
        wqk = np.stack([
            (wqp if t < 4 else wkp)[:, (t % 4) * 128:(t % 4 + 1) * 128]
            .reshape(KC, 128, 128).transpose(1, 0, 2)
            for t in range(8)])
        shard[g] = dict(
            wqk=_bf16(wqk),
            wv=_bf16(Wqkv[:, 2 * INNER:3 * INNER][:, vcols]
                     .reshape(KC, 128, 512).transpose(1, 0, 2)),
            wo=_bf16(Wout[vcols, :].reshape(4, 128, D).transpose(1, 0, 2)),
        )
    in_maps = []
    for c in range(8):
        b, g = c // 2, c % 2
        in_maps.append(dict(
            xT=_bf16(x[b].T), cc=CC, ssw=SSsw, perm=P, **shard[g]))
    return in_maps


def kernel(x, Wqkv, Wout, bout):
    bout = np.asarray(bout, np.float32)
    nc = _get_module()
    in_maps = make_in_maps(x, Wqkv, Wout)
    res = run_bass_kernel_spmd(nc, in_maps, core_ids=list(range(8)))
    ys = [r["y"] for r in res.results]
    out = np.stack([ys[2 * b] + ys[2 * b + 1] + bout for b in range(B)])
    return out.astype(np.float32)
